# revision 23
# baseline (speedup 1.0000x reference)
"""Trainium2 Bass kernel for PhysicsInformedNN (Navier-Stokes PINN residuals).

Computes (u, v, p, f_u, f_v) for N=262144 collocation points through an
8-layer tanh MLP by forward-propagating a 13-channel Taylor jet
(value, 3 first derivs, 5 second derivs, 4 third derivs) per neuron.

Data-parallel over 8 NeuronCores. Per core: points packed into
supertiles of 6 groups x 512 points; the 6 groups share one
block-diagonal [120,120] weight matmul per channel.
"""

import sys
from contextlib import ExitStack

import numpy as np

for _p in ("/opt/trn_rl_repo",):
    if _p not in sys.path:
        sys.path.insert(0, _p)

N_POINTS = 262144
N_CORES = 8
PPC = N_POINTS // N_CORES  # 32768 points per core
WIDTH = 20
N_HID = 7
G = 6          # groups in block-diagonal batch
NPT = 512      # points per group chunk (= fp32 PSUM bank width)
SUPER = G * NPT            # 3072 points per supertile
NS = -(-PPC // SUPER)      # 11 supertiles per core
PADPC = NS * SUPER         # 33792 padded points per core
KDIM = G * WIDTH           # 120
# output-stage PSUM rows: blocks at quadrant starts 0/32/64/96 because
# HW requires compute-op APs to start at partition 0/32/64/96
M_OUT = 108

# jet channel ids
(VAL, CH_X, CH_Y, CH_T, CH_XX, CH_XY, CH_YY, CH_XT, CH_YT,
 CH_XXX, CH_XXY, CH_XYY, CH_YYY) = range(13)
DERIV_CHS = list(range(1, 13))
# channels feeding each output-stage PSUM tile
OUT1_CHS = [CH_X, CH_Y, CH_XX, CH_XY, CH_YY]
OUT2_CHS = [VAL, CH_X, CH_Y, CH_XT, CH_YT, CH_XXX, CH_XXY, CH_XYY, CH_YYY]


def build_host_consts(W_in, b_in, W_hid, b_hid, W_out, b_out, lb, ub,
                      lambda_1, lambda_2):
    f32 = np.float32
    W_in = np.asarray(W_in, f32)
    b_in = np.asarray(b_in, f32)
    W_hid = np.asarray(W_hid, f32)
    b_hid = np.asarray(b_hid, f32)
    W_out = np.asarray(W_out, f32)
    b_out = np.asarray(b_out, f32)
    lb = np.asarray(lb, f32)
    ub = np.asarray(ub, f32)

    # fold normalization h = (z - lb) / (ub - lb) into layer 1
    s = (1.0 / (ub - lb)).astype(f32)
    c0 = (-lb * s).astype(f32)
    Wz = (W_in * s[:, None]).astype(f32)           # [3, 20]
    bz = (c0 @ W_in + b_in).astype(f32)            # [20]

    l1_lhsT = np.zeros((3 * G, KDIM), f32)
    hid_lhsT = np.zeros((N_HID, KDIM, KDIM), f32)
    for g in range(G):
        l1_lhsT[3 * g:3 * g + 3, WIDTH * g:WIDTH * (g + 1)] = Wz
        for l in range(N_HID):
            hid_lhsT[l, WIDTH * g:WIDTH * (g + 1),
                     WIDTH * g:WIDTH * (g + 1)] = W_hid[l]

    bias_tile = np.zeros((KDIM, 8), f32)
    bias_tile[:, 0] = np.tile(bz, G)
    for l in range(N_HID):
        bias_tile[:, 1 + l] = np.tile(b_hid[l], G)

    # layer-1 first-deriv constants (A1 is point-independent, A2 = A3 = 0)
    cx, cy, ct = Wz[0], Wz[1], Wz[2]
    cvecs = [cx, cy, ct,
             cx * cx, cx * cy, cy * cy, cx * ct, cy * ct,
             cx * cx * cx, cx * cx * cy, cx * cy * cy, cy * cy * cy]
    c_tile = np.stack([np.tile(v, G) for v in cvecs], axis=1).astype(f32)

    l1v = float(np.asarray(lambda_1).reshape(-1)[0])
    l2v = float(np.asarray(lambda_2).reshape(-1)[0])
    wpsi, wp = W_out[:, 0], W_out[:, 1]

    # po1 rows: 0:12 [u,u] | 32:44 [v,v] | 64:76 [psi_xy, psi_xx]
    #           | 96:108 [psi_yy, psi_xy]
    # po2 rows: 0:6 u | 32:38 v | 64:70 p_lin | 96:108 [fu_lin, fv_lin]
    # fu_lin = psi_yt + p_x - l2*(psi_xxy + psi_yyy)
    # fv_lin = -psi_xt + p_y + l2*(psi_xxx + psi_xyy)
    o1_lhsT = np.zeros((13, KDIM, M_OUT), f32)
    o2_lhsT = np.zeros((13, KDIM, M_OUT), f32)

    def place(arr, ch, row0, vec):
        for g in range(G):
            arr[ch, WIDTH * g:WIDTH * (g + 1), row0 + g] += vec

    place(o1_lhsT, CH_Y, 0, wpsi)        # u
    place(o1_lhsT, CH_Y, 6, wpsi)        # u
    place(o1_lhsT, CH_X, 32, -wpsi)      # v
    place(o1_lhsT, CH_X, 38, -wpsi)      # v
    place(o1_lhsT, CH_XY, 64, wpsi)      # psi_xy
    place(o1_lhsT, CH_XX, 70, wpsi)      # psi_xx
    place(o1_lhsT, CH_YY, 96, wpsi)      # psi_yy
    place(o1_lhsT, CH_XY, 102, wpsi)     # psi_xy

    place(o2_lhsT, CH_Y, 0, wpsi)        # u
    place(o2_lhsT, CH_X, 32, -wpsi)      # v
    place(o2_lhsT, VAL, 64, wp)          # p (bias added later)
    place(o2_lhsT, CH_YT, 96, wpsi)      # fu_lin
    place(o2_lhsT, CH_X, 96, wp)
    place(o2_lhsT, CH_XXY, 96, -l2v * wpsi)
    place(o2_lhsT, CH_YYY, 96, -l2v * wpsi)
    place(o2_lhsT, CH_XT, 102, -wpsi)    # fv_lin
    place(o2_lhsT, CH_Y, 102, wp)
    place(o2_lhsT, CH_XXX, 102, l2v * wpsi)
    place(o2_lhsT, CH_XYY, 102, l2v * wpsi)

    lam_vec = np.zeros((12, 1), f32)
    lam_vec[0:6, 0] = l1v
    lam_vec[6:12, 0] = -l1v

    return dict(l1_lhsT=l1_lhsT, hid_lhsT=hid_lhsT, bias_tile=bias_tile,
                c_tile=c_tile, o1_lhsT=o1_lhsT, o2_lhsT=o2_lhsT,
                lam_vec=lam_vec, p_bias=float(b_out[1]))


def build_program(p_bias, ns=NS):
    import concourse.bacc as bacc
    import concourse.bass as bass
    import concourse.tile as tile
    from concourse import mybir

    f32 = mybir.dt.float32
    AF = mybir.ActivationFunctionType
    OP = mybir.AluOpType

    nc = bacc.Bacc("TRN2", target_bir_lowering=False, debug=False)

    xyz_d = nc.dram_tensor("xyz", [ns, 3 * G, NPT], f32, kind="ExternalInput")
    l1w_d = nc.dram_tensor("l1_lhsT", [3 * G, KDIM], f32, kind="ExternalInput")
    hw_d = nc.dram_tensor("hid_lhsT", [N_HID, KDIM, KDIM], f32,
                          kind="ExternalInput")
    o1_d = nc.dram_tensor("o1_lhsT", [13, KDIM, M_OUT], f32,
                          kind="ExternalInput")
    o2_d = nc.dram_tensor("o2_lhsT", [13, KDIM, M_OUT], f32,
                          kind="ExternalInput")
    bias_d = nc.dram_tensor("bias_tile", [KDIM, 8], f32, kind="ExternalInput")
    c_d = nc.dram_tensor("c_tile", [KDIM, 12], f32, kind="ExternalInput")
    lam_d = nc.dram_tensor("lam_vec", [12, 1], f32, kind="ExternalInput")
    u_d = nc.dram_tensor("u_out", [ns, G, NPT], f32, kind="ExternalOutput")
    v_d = nc.dram_tensor("v_out", [ns, G, NPT], f32, kind="ExternalOutput")
    p_d = nc.dram_tensor("p_out", [ns, G, NPT], f32, kind="ExternalOutput")
    fu_d = nc.dram_tensor("fu_out", [ns, G, NPT], f32, kind="ExternalOutput")
    fv_d = nc.dram_tensor("fv_out", [ns, G, NPT], f32, kind="ExternalOutput")

    with tile.TileContext(nc) as tc, ExitStack() as ctx:
        dma = nc.sync.dma_start
        act = nc.scalar.activation
        tt = nc.vector.tensor_tensor
        ts = nc.vector.tensor_scalar
        stt = nc.vector.scalar_tensor_tensor
        mm = nc.tensor.matmul

        # persistent weights / constants (one pool, never rotated)
        wpool = ctx.enter_context(tc.tile_pool(name="wpool", bufs=1))
        l1w = wpool.tile([3 * G, KDIM], f32, name="l1w")
        dma(l1w[:], l1w_d[:])
        hws = []
        for l in range(N_HID):
            w = wpool.tile([KDIM, KDIM], f32, name=f"hw{l}")
            dma(w[:], hw_d[l])
            hws.append(w)
        ow1, ow2 = {}, {}
        for ch in OUT1_CHS:
            w = wpool.tile([KDIM, M_OUT], f32, name=f"ow1_{ch}")
            dma(w[:], o1_d[ch])
            ow1[ch] = w
        for ch in OUT2_CHS:
            w = wpool.tile([KDIM, M_OUT], f32, name=f"ow2_{ch}")
            dma(w[:], o2_d[ch])
            ow2[ch] = w
        biases = wpool.tile([KDIM, 8], f32, name="biases")
        dma(biases[:], bias_d[:])
        ctile = wpool.tile([KDIM, 12], f32, name="ctile")
        dma(ctile[:], c_d[:])
        lam = wpool.tile([12, 1], f32, name="lam")
        dma(lam[:], lam_d[:])

        xin = ctx.enter_context(tc.tile_pool(name="xin", bufs=2))
        bpool = ctx.enter_context(tc.tile_pool(name="bch", bufs=2))
        tpool = ctx.enter_context(tc.tile_pool(name="tmp", bufs=1))
        o12 = ctx.enter_context(tc.tile_pool(name="o12", bufs=2))
        o6 = ctx.enter_context(tc.tile_pool(name="o6", bufs=2))
        psA = ctx.enter_context(
            tc.tile_pool(name="psA", bufs=1, space=bass.MemorySpace.PSUM))
        psB = ctx.enter_context(
            tc.tile_pool(name="psB", bufs=2, space=bass.MemorySpace.PSUM))
        psO = ctx.enter_context(
            tc.tile_pool(name="psO", bufs=1, space=bass.MemorySpace.PSUM))

        def btile(ch):
            return bpool.tile([KDIM, NPT], f32, name=f"B{ch}")

        def ttile(nm, bufs=None):
            return tpool.tile([KDIM, NPT], f32, name=nm, bufs=bufs)

        def hidden_layer(l, Bp):
            W = hws[l]
            pa = {}
            for ch, nm in ((VAL, "paV"), (CH_X, "paX"), (CH_Y, "paY"),
                           (CH_T, "paT")):
                pa[ch] = psA.tile([KDIM, NPT], f32, name=nm)
                mm(pa[ch][:], W[:], Bp[ch][:], start=True, stop=True)
            for ch in (CH_XX, CH_XY, CH_YY, CH_XT, CH_YT,
                       CH_XXX, CH_XXY, CH_XYY, CH_YYY):
                pa[ch] = psB.tile([KDIM, NPT], f32, name="pb")
                mm(pa[ch][:], W[:], Bp[ch][:], start=True, stop=True)

            Bn = {}
            t0 = btile(VAL)
            act(t0[:], pa[VAL][:], AF.Tanh, bias=biases[:, 1 + l:2 + l])
            Bn[VAL] = t0
            p2 = ttile("p2")
            act(p2[:], t0[:], AF.Square)
            mneg = ttile("mneg")
            act(mneg[:], t0[:], AF.Copy, bias=0.0, scale=-2.0)
            qq = ttile("qq")
            act(qq[:], p2[:], AF.Copy, bias=-2.0, scale=6.0)

            f1 = ttile("f1")
            ts(f1[:], p2[:], -1.0, 1.0, OP.mult, OP.add)
            # first derivs: B1j = f1 * A1j
            for ch in (CH_X, CH_Y, CH_T):
                Bn[ch] = btile(ch)
                tt(Bn[ch][:], f1[:], pa[ch][:], OP.mult)
            # w_j = -2 t0 * A1j
            w = {}
            for ch in (CH_X, CH_Y, CH_T):
                w[ch] = ttile(f"w{ch}")
                tt(w[ch][:], mneg[:], pa[ch][:], OP.mult)
            # pure cubic terms: f3*A1a*A1b*A1c  (f3 = q*f1)
            pxx = ttile("pxx")
            tt(pxx[:], Bn[CH_X][:], pa[CH_X][:], OP.mult)
            pyy = ttile("pyy")
            tt(pyy[:], Bn[CH_Y][:], pa[CH_Y][:], OP.mult)
            sx = ttile("sx")
            tt(sx[:], qq[:], pxx[:], OP.mult)
            sy = ttile("sy")
            tt(sy[:], qq[:], pyy[:], OP.mult)
            cub = {}
            for ch, (sv, pach) in ((CH_XXX, (sx, CH_X)), (CH_XXY, (sx, CH_Y)),
                                   (CH_XYY, (sy, CH_X)), (CH_YYY, (sy, CH_Y))):
                cub[ch] = ttile(f"cub{ch}")
                tt(cub[ch][:], sv[:], pa[pach][:], OP.mult)
            # v_jk = f1 * A2jk
            v = {}
            for ch in (CH_XX, CH_XY, CH_YY, CH_XT, CH_YT):
                v[ch] = ttile(f"v{ch}")
                tt(v[ch][:], f1[:], pa[ch][:], OP.mult)
            # cross_jk = (f1 A1j) * (m A1k) = f2 A1j A1k
            cross_src = {CH_XX: (CH_X, CH_X), CH_XY: (CH_X, CH_Y),
                         CH_YY: (CH_Y, CH_Y), CH_XT: (CH_X, CH_T),
                         CH_YT: (CH_Y, CH_T)}
            for ch in (CH_XX, CH_XY, CH_YY, CH_XT, CH_YT):
                ju, kw = cross_src[ch]
                cr = ttile("cr", bufs=2)
                tt(cr[:], Bn[ju][:], w[kw][:], OP.mult)
                Bn[ch] = btile(ch)
                tt(Bn[ch][:], v[ch][:], cr[:], OP.add)
            # second-order mix for third derivs
            g = {}
            g[CH_XXX] = ttile("gxxx")
            stt(g[CH_XXX][:], v[CH_XX][:], 3.0, w[CH_X][:], OP.mult, OP.mult)
            g[CH_YYY] = ttile("gyyy")
            stt(g[CH_YYY][:], v[CH_YY][:], 3.0, w[CH_Y][:], OP.mult, OP.mult)
            ga = ttile("ga")
            stt(ga[:], v[CH_XY][:], 2.0, w[CH_X][:], OP.mult, OP.mult)
            gb = ttile("gb")
            tt(gb[:], v[CH_XX][:], w[CH_Y][:], OP.mult)
            g[CH_XXY] = ttile("gxxy")
            tt(g[CH_XXY][:], ga[:], gb[:], OP.add)
            gc = ttile("gc")
            stt(gc[:], v[CH_XY][:], 2.0, w[CH_Y][:], OP.mult, OP.mult)
            gd = ttile("gd")
            tt(gd[:], v[CH_YY][:], w[CH_X][:], OP.mult)
            g[CH_XYY] = ttile("gxyy")
            tt(g[CH_XYY][:], gc[:], gd[:], OP.add)
            # B3 = cub + g + f1*A3
            for ch in (CH_XXX, CH_XXY, CH_XYY, CH_YYY):
                t3 = ttile("t3", bufs=2)
                tt(t3[:], f1[:], pa[ch][:], OP.mult)
                h = ttile("h3", bufs=2)
                tt(h[:], cub[ch][:], g[ch][:], OP.add)
                Bn[ch] = btile(ch)
                tt(Bn[ch][:], h[:], t3[:], OP.add)
            return Bn

        for sidx in range(ns):
            xt = xin.tile([3 * G, NPT], f32, name="xt")
            dma(xt[:], xyz_d[sidx])

            # ---- layer 1: A1 = const per partition, A2 = A3 = 0 ----
            pa0 = psA.tile([KDIM, NPT], f32, name="paV")
            mm(pa0[:], l1w[:], xt[:], start=True, stop=True)
            B = {}
            t0 = btile(VAL)
            act(t0[:], pa0[:], AF.Tanh, bias=biases[:, 0:1])
            B[VAL] = t0
            p2 = ttile("p2")
            act(p2[:], t0[:], AF.Square)
            mneg = ttile("mneg")
            act(mneg[:], t0[:], AF.Copy, bias=0.0, scale=-2.0)
            qq = ttile("qq")
            act(qq[:], p2[:], AF.Copy, bias=-2.0, scale=6.0)
            f1 = ttile("f1")
            ts(f1[:], p2[:], -1.0, 1.0, OP.mult, OP.add)
            f2 = ttile("f2")
            tt(f2[:], mneg[:], f1[:], OP.mult)
            f3 = ttile("f3")
            tt(f3[:], qq[:], f1[:], OP.mult)
            for k, ch in enumerate(DERIV_CHS):
                src = f1 if ch <= CH_T else (f2 if ch <= CH_YT else f3)
                B[ch] = btile(ch)
                ts(B[ch][:], src[:], ctile[:, k:k + 1], None, OP.mult)

            # ---- hidden layers ----
            for l in range(N_HID):
                B = hidden_layer(l, B)

            # ---- output stage (blocks at quadrant starts 0/32/64/96) ----
            po1 = psO.tile([M_OUT, NPT], f32, name="po1")
            for i, ch in enumerate(OUT1_CHS):
                mm(po1[:], ow1[ch][:], B[ch][:],
                   start=(i == 0), stop=(i == len(OUT1_CHS) - 1))
            po2 = psO.tile([M_OUT, NPT], f32, name="po2")
            for i, ch in enumerate(OUT2_CHS):
                mm(po2[:], ow2[ch][:], B[ch][:],
                   start=(i == 0), stop=(i == len(OUT2_CHS) - 1))
            a1 = o12.tile([12, NPT], f32, name="a1")
            act(a1[:], po1[0:12, :], AF.Copy, bias=0.0, scale=1.0)
            a2 = o12.tile([12, NPT], f32, name="a2")
            act(a2[:], po1[32:44, :], AF.Copy, bias=0.0, scale=1.0)
            pl1 = o12.tile([12, NPT], f32, name="pl1")
            tt(pl1[:], a1[:], po1[64:76, :], OP.mult)
            pl2 = o12.tile([12, NPT], f32, name="pl2")
            tt(pl2[:], a2[:], po1[96:108, :], OP.mult)
            dd = o12.tile([12, NPT], f32, name="dd")
            tt(dd[:], pl1[:], pl2[:], OP.add)
            ff = o12.tile([12, NPT], f32, name="ff")
            stt(ff[:], dd[:], lam[:], po2[96:108, :], OP.mult, OP.add)
            pp = o6.tile([6, NPT], f32, name="pp")
            act(pp[:], po2[64:70, :], AF.Copy, bias=float(p_bias), scale=1.0)
            uu = o6.tile([6, NPT], f32, name="uu")
            act(uu[:], po2[0:6, :], AF.Copy, bias=0.0, scale=1.0)
            vv = o6.tile([6, NPT], f32, name="vv")
            act(vv[:], po2[32:38, :], AF.Copy, bias=0.0, scale=1.0)

            dma(u_d[sidx], uu[:])
            dma(v_d[sidx], vv[:])
            dma(p_d[sidx], pp[:])
            dma(fu_d[sidx], ff[0:6, :])
            dma(fv_d[sidx], ff[6:12, :])

    nc.compile()
    return nc


def make_in_maps(inputs, consts, ns=NS):
    x = np.asarray(inputs["x"], np.float32).reshape(-1)
    y = np.asarray(inputs["y"], np.float32).reshape(-1)
    t = np.asarray(inputs["t"], np.float32).reshape(-1)
    padpc = ns * SUPER
    shared = {k: consts[k] for k in ("l1_lhsT", "hid_lhsT", "o1_lhsT",
                                     "o2_lhsT", "bias_tile", "c_tile",
                                     "lam_vec")}
    in_maps = []
    for c in range(N_CORES):
        sl = slice(c * PPC, (c + 1) * PPC)

        def lay(vec):
            out = np.zeros((padpc,), np.float32)
            seg = vec[sl]
            out[:seg.shape[0]] = seg[:padpc]
            return out.reshape(ns, G, NPT)

        xyz = np.zeros((ns, 3 * G, NPT), np.float32)
        xyz[:, 0::3, :] = lay(x)
        xyz[:, 1::3, :] = lay(y)
        xyz[:, 2::3, :] = lay(t)
        in_maps.append({"xyz": xyz, **shared})
    return in_maps


def kernel(**inputs):
    consts = build_host_consts(
        inputs["W_in"], inputs["b_in"], inputs["W_hid"], inputs["b_hid"],
        inputs["W_out"], inputs["b_out"], inputs["lb"], inputs["ub"],
        inputs["lambda_1"], inputs["lambda_2"])
    nc = build_program(consts["p_bias"])
    in_maps = make_in_maps(inputs, consts)

    from concourse.bass_utils import run_bass_kernel_spmd
    res = run_bass_kernel_spmd(nc, in_maps, list(range(N_CORES)))

    outs = []
    for name in ("u_out", "v_out", "p_out", "fu_out", "fv_out"):
        full = np.concatenate(
            [np.asarray(res.results[c][name]).reshape(-1)[:PPC]
             for c in range(N_CORES)])
        outs.append(np.ascontiguousarray(full[:, None], dtype=np.float32))
    return tuple(outs)


# revision 27
# speedup vs baseline: 1.7035x; 1.7035x over previous
"""Trainium2 Bass kernel for PhysicsInformedNN (Navier-Stokes PINN residuals).

Computes (u, v, p, f_u, f_v) for N=262144 collocation points through an
8-layer tanh MLP by forward-propagating a 13-channel Taylor jet
(value, 3 first derivs, 5 second derivs, 4 third derivs) per neuron.

Data-parallel over 8 NeuronCores. Per core: points packed into
supertiles of 6 groups x 512 points; the 6 groups share one
block-diagonal [120,120] weight matmul per channel.

v2: fp16 matmul inputs (PE 1 cycle/row vs 4 for fp32), fp16 SBUF
elementwise tiles (DVE 4x perf mode), term-splitting (additive jet
pieces fed as separate accumulating matmuls instead of DVE adds),
and elementwise work spread over ACT / DVE / GpSimd engines.
"""

import sys
from contextlib import ExitStack

import numpy as np

for _p in ("/opt/trn_rl_repo",):
    if _p not in sys.path:
        sys.path.insert(0, _p)

N_POINTS = 262144
N_CORES = 8
PPC = N_POINTS // N_CORES  # 32768 points per core
WIDTH = 20
N_HID = 7
G = 6          # groups in block-diagonal batch
NPT = 512      # points per group chunk (= fp32 PSUM bank width)
SUPER = G * NPT            # 3072 points per supertile
NS = -(-PPC // SUPER)      # 11 supertiles per core
PADPC = NS * SUPER         # 33792 padded points per core
KDIM = G * WIDTH           # 120
# output-stage PSUM rows: blocks at quadrant starts 0/32/64/96 because
# HW requires compute-op APs to start at partition 0/32/64/96
M_OUT = 108

# jet channel ids
(VAL, CH_X, CH_Y, CH_T, CH_XX, CH_XY, CH_YY, CH_XT, CH_YT,
 CH_XXX, CH_XXY, CH_XYY, CH_YYY) = range(13)
DERIV_CHS = list(range(1, 13))
CH2 = (CH_XX, CH_XY, CH_YY, CH_XT, CH_YT)
CH3 = (CH_XXX, CH_XXY, CH_XYY, CH_YYY)
# channels feeding each output-stage PSUM tile
OUT1_CHS = [CH_X, CH_Y, CH_XX, CH_XY, CH_YY]
OUT2_CHS = [VAL, CH_X, CH_Y, CH_XT, CH_YT, CH_XXX, CH_XXY, CH_XYY, CH_YYY]


def build_host_consts(W_in, b_in, W_hid, b_hid, W_out, b_out, lb, ub,
                      lambda_1, lambda_2):
    f32 = np.float32
    W_in = np.asarray(W_in, f32)
    b_in = np.asarray(b_in, f32)
    W_hid = np.asarray(W_hid, f32)
    b_hid = np.asarray(b_hid, f32)
    W_out = np.asarray(W_out, f32)
    b_out = np.asarray(b_out, f32)
    lb = np.asarray(lb, f32)
    ub = np.asarray(ub, f32)

    # fold normalization h = (z - lb) / (ub - lb) into layer 1
    s = (1.0 / (ub - lb)).astype(f32)
    c0 = (-lb * s).astype(f32)
    Wz = (W_in * s[:, None]).astype(f32)           # [3, 20]
    bz = (c0 @ W_in + b_in).astype(f32)            # [20]

    l1_lhsT = np.zeros((3 * G, KDIM), f32)
    hid_lhsT = np.zeros((N_HID, KDIM, KDIM), f32)
    for g in range(G):
        l1_lhsT[3 * g:3 * g + 3, WIDTH * g:WIDTH * (g + 1)] = Wz
        for l in range(N_HID):
            hid_lhsT[l, WIDTH * g:WIDTH * (g + 1),
                     WIDTH * g:WIDTH * (g + 1)] = W_hid[l]

    bias_tile = np.zeros((KDIM, 8), f32)
    bias_tile[:, 0] = np.tile(bz, G)
    for l in range(N_HID):
        bias_tile[:, 1 + l] = np.tile(b_hid[l], G)

    # layer-1 first-deriv constants (A1 is point-independent, A2 = A3 = 0)
    cx, cy, ct = Wz[0], Wz[1], Wz[2]
    cvecs = [cx, cy, ct,
             cx * cx, cx * cy, cy * cy, cx * ct, cy * ct,
             cx * cx * cx, cx * cx * cy, cx * cy * cy, cy * cy * cy]
    c_tile = np.stack([np.tile(v, G) for v in cvecs], axis=1).astype(f32)

    l1v = float(np.asarray(lambda_1).reshape(-1)[0])
    l2v = float(np.asarray(lambda_2).reshape(-1)[0])
    wpsi, wp = W_out[:, 0], W_out[:, 1]

    # po1 rows: 0:12 [u,u] | 32:44 [v,v] | 64:76 [psi_xy, psi_xx]
    #           | 96:108 [psi_yy, psi_xy]
    # po2 rows: 0:6 u | 32:38 v | 64:70 p_lin | 96:108 [fu_lin, fv_lin]
    # fu_lin = psi_yt + p_x - l2*(psi_xxy + psi_yyy)
    # fv_lin = -psi_xt + p_y + l2*(psi_xxx + psi_xyy)
    o1_lhsT = np.zeros((13, KDIM, M_OUT), f32)
    o2_lhsT = np.zeros((13, KDIM, M_OUT), f32)

    def place(arr, ch, row0, vec):
        for g in range(G):
            arr[ch, WIDTH * g:WIDTH * (g + 1), row0 + g] += vec

    place(o1_lhsT, CH_Y, 0, wpsi)        # u
    place(o1_lhsT, CH_Y, 6, wpsi)        # u
    place(o1_lhsT, CH_X, 32, -wpsi)      # v
    place(o1_lhsT, CH_X, 38, -wpsi)      # v
    place(o1_lhsT, CH_XY, 64, wpsi)      # psi_xy
    place(o1_lhsT, CH_XX, 70, wpsi)      # psi_xx
    place(o1_lhsT, CH_YY, 96, wpsi)      # psi_yy
    place(o1_lhsT, CH_XY, 102, wpsi)     # psi_xy

    place(o2_lhsT, CH_Y, 0, wpsi)        # u
    place(o2_lhsT, CH_X, 32, -wpsi)      # v
    place(o2_lhsT, VAL, 64, wp)          # p (bias added later)
    place(o2_lhsT, CH_YT, 96, wpsi)      # fu_lin
    place(o2_lhsT, CH_X, 96, wp)
    place(o2_lhsT, CH_XXY, 96, -l2v * wpsi)
    place(o2_lhsT, CH_YYY, 96, -l2v * wpsi)
    place(o2_lhsT, CH_XT, 102, -wpsi)    # fv_lin
    place(o2_lhsT, CH_Y, 102, wp)
    place(o2_lhsT, CH_XXX, 102, l2v * wpsi)
    place(o2_lhsT, CH_XYY, 102, l2v * wpsi)

    lam_vec = np.zeros((12, 1), f32)
    lam_vec[0:6, 0] = l1v
    lam_vec[6:12, 0] = -l1v

    f16 = np.float16
    return dict(l1_lhsT=l1_lhsT.astype(f16), hid_lhsT=hid_lhsT.astype(f16),
                bias_tile=bias_tile, c_tile=c_tile,
                o1_lhsT=o1_lhsT.astype(f16), o2_lhsT=o2_lhsT.astype(f16),
                lam_vec=lam_vec, p_bias=float(b_out[1]))


def build_program(p_bias, ns=NS):
    import concourse.bacc as bacc
    import concourse.bass as bass
    import concourse.tile as tile
    from concourse import mybir

    f32 = mybir.dt.float32
    f16 = mybir.dt.float16
    AF = mybir.ActivationFunctionType
    OP = mybir.AluOpType

    nc = bacc.Bacc("TRN2", target_bir_lowering=False, debug=False)

    xyz_d = nc.dram_tensor("xyz", [ns, 3 * G, NPT], f16, kind="ExternalInput")
    l1w_d = nc.dram_tensor("l1_lhsT", [3 * G, KDIM], f16, kind="ExternalInput")
    hw_d = nc.dram_tensor("hid_lhsT", [N_HID, KDIM, KDIM], f16,
                          kind="ExternalInput")
    o1_d = nc.dram_tensor("o1_lhsT", [13, KDIM, M_OUT], f16,
                          kind="ExternalInput")
    o2_d = nc.dram_tensor("o2_lhsT", [13, KDIM, M_OUT], f16,
                          kind="ExternalInput")
    bias_d = nc.dram_tensor("bias_tile", [KDIM, 8], f32, kind="ExternalInput")
    c_d = nc.dram_tensor("c_tile", [KDIM, 12], f32, kind="ExternalInput")
    lam_d = nc.dram_tensor("lam_vec", [12, 1], f32, kind="ExternalInput")
    u_d = nc.dram_tensor("u_out", [ns, G, NPT], f32, kind="ExternalOutput")
    v_d = nc.dram_tensor("v_out", [ns, G, NPT], f32, kind="ExternalOutput")
    p_d = nc.dram_tensor("p_out", [ns, G, NPT], f32, kind="ExternalOutput")
    fu_d = nc.dram_tensor("fu_out", [ns, G, NPT], f32, kind="ExternalOutput")
    fv_d = nc.dram_tensor("fv_out", [ns, G, NPT], f32, kind="ExternalOutput")

    with tile.TileContext(nc) as tc, ExitStack() as ctx:
        dma = nc.sync.dma_start
        act = nc.scalar.activation
        tt = nc.vector.tensor_tensor
        ts = nc.vector.tensor_scalar
        stt = nc.vector.scalar_tensor_tensor
        gtt = nc.gpsimd.tensor_tensor
        mm = nc.tensor.matmul

        # persistent weights / constants (one pool, never rotated)
        wpool = ctx.enter_context(tc.tile_pool(name="wpool", bufs=1))
        l1w = wpool.tile([3 * G, KDIM], f16, name="l1w")
        dma(l1w[:], l1w_d[:])
        hws = []
        for l in range(N_HID):
            w = wpool.tile([KDIM, KDIM], f16, name=f"hw{l}")
            dma(w[:], hw_d[l])
            hws.append(w)
        ow1, ow2 = {}, {}
        for ch in OUT1_CHS:
            w = wpool.tile([KDIM, M_OUT], f16, name=f"ow1_{ch}")
            dma(w[:], o1_d[ch])
            ow1[ch] = w
        for ch in OUT2_CHS:
            w = wpool.tile([KDIM, M_OUT], f16, name=f"ow2_{ch}")
            dma(w[:], o2_d[ch])
            ow2[ch] = w
        biases = wpool.tile([KDIM, 8], f32, name="biases")
        dma(biases[:], bias_d[:])
        ctile = wpool.tile([KDIM, 12], f32, name="ctile")
        dma(ctile[:], c_d[:])
        lam = wpool.tile([12, 1], f32, name="lam")
        dma(lam[:], lam_d[:])

        xin = ctx.enter_context(tc.tile_pool(name="xin", bufs=2))
        bpool = ctx.enter_context(tc.tile_pool(name="bch", bufs=2))
        tpool = ctx.enter_context(tc.tile_pool(name="tmp", bufs=1))
        o12 = ctx.enter_context(tc.tile_pool(name="o12", bufs=2))
        o6 = ctx.enter_context(tc.tile_pool(name="o6", bufs=2))
        psA = ctx.enter_context(
            tc.tile_pool(name="psA", bufs=1, space=bass.MemorySpace.PSUM))
        psB = ctx.enter_context(
            tc.tile_pool(name="psB", bufs=2, space=bass.MemorySpace.PSUM))
        psO = ctx.enter_context(
            tc.tile_pool(name="psO", bufs=1, space=bass.MemorySpace.PSUM))

        def btile(nm):
            return bpool.tile([KDIM, NPT], f16, name=nm)

        def ttile(nm):
            return tpool.tile([KDIM, NPT], f16, name=nm)

        def mm_acc(ps, W, pieces):
            n = len(pieces)
            for i, p in enumerate(pieces):
                mm(ps[:], W[:], p[:], start=(i == 0), stop=(i == n - 1))

        def hidden_layer(l, Bp):
            W = hws[l]
            pa = {}
            for ch, nm in ((VAL, "paV"), (CH_X, "paX"), (CH_Y, "paY"),
                           (CH_T, "paT")):
                pa[ch] = psA.tile([KDIM, NPT], f32, name=nm)
                mm_acc(pa[ch], W, Bp[ch])
            for ch in CH2 + CH3:
                pa[ch] = psB.tile([KDIM, NPT], f32, name="pb")
                mm_acc(pa[ch], W, Bp[ch])

            # ACT engine: tanh + fp16 SBUF copies of hot PSUM channels
            t0 = btile("t0")
            act(t0[:], pa[VAL][:], AF.Tanh, bias=biases[:, 1 + l:2 + l])
            qx = ttile("qx")
            act(qx[:], pa[CH_X][:], AF.Copy, bias=0.0, scale=1.0)
            qy = ttile("qy")
            act(qy[:], pa[CH_Y][:], AF.Copy, bias=0.0, scale=1.0)
            qt = ttile("qt")
            act(qt[:], pa[CH_T][:], AF.Copy, bias=0.0, scale=1.0)
            p2 = ttile("p2")
            tt(p2[:], t0[:], t0[:], OP.mult)
            mneg = ttile("mneg")
            ts(mneg[:], t0[:], -2.0, None, OP.mult)
            qq = ttile("qq")
            ts(qq[:], p2[:], 6.0, -2.0, OP.mult, OP.add)

            # DVE fp16 SBUF chain
            f1 = ttile("f1")
            ts(f1[:], p2[:], -1.0, 1.0, OP.mult, OP.add)
            B1x = btile("B1x")
            tt(B1x[:], f1[:], qx[:], OP.mult)
            B1y = btile("B1y")
            tt(B1y[:], f1[:], qy[:], OP.mult)
            B1t = btile("B1t")
            tt(B1t[:], f1[:], qt[:], OP.mult)
            wx = ttile("wx")
            tt(wx[:], mneg[:], qx[:], OP.mult)
            wy = ttile("wy")
            tt(wy[:], mneg[:], qy[:], OP.mult)
            wt = ttile("wt")
            tt(wt[:], mneg[:], qt[:], OP.mult)
            pxx = ttile("pxx")
            tt(pxx[:], B1x[:], qx[:], OP.mult)
            pyy = ttile("pyy")
            tt(pyy[:], B1y[:], qy[:], OP.mult)
            sx = ttile("sx")
            tt(sx[:], qq[:], pxx[:], OP.mult)
            sy = ttile("sy")
            tt(sy[:], qq[:], pyy[:], OP.mult)
            cub = {}
            for ch, (sv, qv) in ((CH_XXX, (sx, qx)), (CH_XXY, (sx, qy)),
                                 (CH_XYY, (sy, qx)), (CH_YYY, (sy, qy))):
                cub[ch] = btile(f"cub{ch}")
                tt(cub[ch][:], sv[:], qv[:], OP.mult)
            cr = {}
            for ch, (bv, wv) in ((CH_XX, (B1x, wx)), (CH_XY, (B1x, wy)),
                                 (CH_YY, (B1y, wy)), (CH_XT, (B1x, wt)),
                                 (CH_YT, (B1y, wt))):
                cr[ch] = btile(f"cr{ch}")
                tt(cr[ch][:], bv[:], wv[:], OP.mult)

            # f1 * pa for second/third-deriv channels (read PSUM directly):
            # the three needed again by g-terms go on DVE, rest on GpSimd
            v = {}
            for ch in (CH_XX, CH_XY, CH_YY):
                v[ch] = btile(f"v{ch}")
                tt(v[ch][:], f1[:], pa[ch][:], OP.mult)
            t3 = {}
            t3[CH_YYY] = btile(f"t3{CH_YYY}")
            tt(t3[CH_YYY][:], f1[:], pa[CH_YYY][:], OP.mult)
            # GpSimd cannot read PSUM: stage via ACT copy to fp16 SBUF
            for ch in (CH_XT, CH_YT):
                qc = ttile(f"q{ch}")
                act(qc[:], pa[ch][:], AF.Copy, bias=0.0, scale=1.0)
                v[ch] = btile(f"v{ch}")
                gtt(v[ch][:], f1[:], qc[:], OP.mult)
            for ch in (CH_XXX, CH_XXY, CH_XYY):
                qc = ttile(f"q{ch}")
                act(qc[:], pa[ch][:], AF.Copy, bias=0.0, scale=1.0)
                t3[ch] = btile(f"t3{ch}")
                gtt(t3[ch][:], f1[:], qc[:], OP.mult)

            gxxx = btile("gxxx")
            stt(gxxx[:], v[CH_XX][:], 3.0, wx[:], OP.mult, OP.mult)
            gyyy = btile("gyyy")
            stt(gyyy[:], v[CH_YY][:], 3.0, wy[:], OP.mult, OP.mult)
            ga = btile("ga")
            stt(ga[:], v[CH_XY][:], 2.0, wx[:], OP.mult, OP.mult)
            gb = btile("gb")
            tt(gb[:], v[CH_XX][:], wy[:], OP.mult)
            gc = btile("gc")
            stt(gc[:], v[CH_XY][:], 2.0, wy[:], OP.mult, OP.mult)
            gd = btile("gd")
            tt(gd[:], v[CH_YY][:], wx[:], OP.mult)

            return {VAL: [t0], CH_X: [B1x], CH_Y: [B1y], CH_T: [B1t],
                    CH_XX: [v[CH_XX], cr[CH_XX]],
                    CH_XY: [v[CH_XY], cr[CH_XY]],
                    CH_YY: [v[CH_YY], cr[CH_YY]],
                    CH_XT: [v[CH_XT], cr[CH_XT]],
                    CH_YT: [v[CH_YT], cr[CH_YT]],
                    CH_XXX: [cub[CH_XXX], gxxx, t3[CH_XXX]],
                    CH_XXY: [cub[CH_XXY], ga, gb, t3[CH_XXY]],
                    CH_XYY: [cub[CH_XYY], gc, gd, t3[CH_XYY]],
                    CH_YYY: [cub[CH_YYY], gyyy, t3[CH_YYY]]}

        for sidx in range(ns):
            xt = xin.tile([3 * G, NPT], f16, name="xt")
            dma(xt[:], xyz_d[sidx])

            # ---- layer 1: A1 = const per partition, A2 = A3 = 0 ----
            pa0 = psA.tile([KDIM, NPT], f32, name="paV")
            mm(pa0[:], l1w[:], xt[:], start=True, stop=True)
            t0 = btile("t0")
            act(t0[:], pa0[:], AF.Tanh, bias=biases[:, 0:1])
            B = {VAL: [t0]}
            p2 = ttile("p2")
            act(p2[:], t0[:], AF.Square)
            mneg = ttile("mneg")
            act(mneg[:], t0[:], AF.Copy, bias=0.0, scale=-2.0)
            qq = ttile("qq")
            act(qq[:], p2[:], AF.Copy, bias=-2.0, scale=6.0)
            f1 = ttile("f1")
            ts(f1[:], p2[:], -1.0, 1.0, OP.mult, OP.add)
            f2 = ttile("f2")
            tt(f2[:], mneg[:], f1[:], OP.mult)
            f3 = ttile("f3")
            tt(f3[:], qq[:], f1[:], OP.mult)
            for k, ch in enumerate(DERIV_CHS):
                src = f1 if ch <= CH_T else (f2 if ch <= CH_YT else f3)
                b = btile(f"L1B{ch}")
                ts(b[:], src[:], ctile[:, k:k + 1], None, OP.mult)
                B[ch] = [b]

            # ---- hidden layers ----
            for l in range(N_HID):
                B = hidden_layer(l, B)

            # ---- output stage (blocks at quadrant starts 0/32/64/96) ----
            po1 = psO.tile([M_OUT, NPT], f32, name="po1")
            p1 = [(ch, p) for ch in OUT1_CHS for p in B[ch]]
            for i, (ch, p) in enumerate(p1):
                mm(po1[:], ow1[ch][:], p[:],
                   start=(i == 0), stop=(i == len(p1) - 1))
            po2 = psO.tile([M_OUT, NPT], f32, name="po2")
            p2l = [(ch, p) for ch in OUT2_CHS for p in B[ch]]
            for i, (ch, p) in enumerate(p2l):
                mm(po2[:], ow2[ch][:], p[:],
                   start=(i == 0), stop=(i == len(p2l) - 1))
            a1 = o12.tile([12, NPT], f32, name="a1")
            act(a1[:], po1[0:12, :], AF.Copy, bias=0.0, scale=1.0)
            a2 = o12.tile([12, NPT], f32, name="a2")
            act(a2[:], po1[32:44, :], AF.Copy, bias=0.0, scale=1.0)
            pl1 = o12.tile([12, NPT], f32, name="pl1")
            tt(pl1[:], a1[:], po1[64:76, :], OP.mult)
            pl2 = o12.tile([12, NPT], f32, name="pl2")
            tt(pl2[:], a2[:], po1[96:108, :], OP.mult)
            dd = o12.tile([12, NPT], f32, name="dd")
            tt(dd[:], pl1[:], pl2[:], OP.add)
            ff = o12.tile([12, NPT], f32, name="ff")
            stt(ff[:], dd[:], lam[:], po2[96:108, :], OP.mult, OP.add)
            pp = o6.tile([6, NPT], f32, name="pp")
            act(pp[:], po2[64:70, :], AF.Copy, bias=float(p_bias), scale=1.0)
            uu = o6.tile([6, NPT], f32, name="uu")
            act(uu[:], po2[0:6, :], AF.Copy, bias=0.0, scale=1.0)
            vv = o6.tile([6, NPT], f32, name="vv")
            act(vv[:], po2[32:38, :], AF.Copy, bias=0.0, scale=1.0)

            dma(u_d[sidx], uu[:])
            dma(v_d[sidx], vv[:])
            dma(p_d[sidx], pp[:])
            dma(fu_d[sidx], ff[0:6, :])
            dma(fv_d[sidx], ff[6:12, :])

    nc.compile()
    return nc


def make_in_maps(inputs, consts, ns=NS):
    x = np.asarray(inputs["x"], np.float32).reshape(-1)
    y = np.asarray(inputs["y"], np.float32).reshape(-1)
    t = np.asarray(inputs["t"], np.float32).reshape(-1)
    padpc = ns * SUPER
    shared = {k: consts[k] for k in ("l1_lhsT", "hid_lhsT", "o1_lhsT",
                                     "o2_lhsT", "bias_tile", "c_tile",
                                     "lam_vec")}
    in_maps = []
    for c in range(N_CORES):
        sl = slice(c * PPC, (c + 1) * PPC)

        def lay(vec):
            out = np.zeros((padpc,), np.float32)
            seg = vec[sl]
            out[:seg.shape[0]] = seg[:padpc]
            return out.reshape(ns, G, NPT)

        xyz = np.zeros((ns, 3 * G, NPT), np.float32)
        xyz[:, 0::3, :] = lay(x)
        xyz[:, 1::3, :] = lay(y)
        xyz[:, 2::3, :] = lay(t)
        in_maps.append({"xyz": xyz.astype(np.float16), **shared})
    return in_maps


def kernel(**inputs):
    consts = build_host_consts(
        inputs["W_in"], inputs["b_in"], inputs["W_hid"], inputs["b_hid"],
        inputs["W_out"], inputs["b_out"], inputs["lb"], inputs["ub"],
        inputs["lambda_1"], inputs["lambda_2"])
    nc = build_program(consts["p_bias"])
    in_maps = make_in_maps(inputs, consts)

    from concourse.bass_utils import run_bass_kernel_spmd
    res = run_bass_kernel_spmd(nc, in_maps, list(range(N_CORES)))

    outs = []
    for name in ("u_out", "v_out", "p_out", "fu_out", "fv_out"):
        full = np.concatenate(
            [np.asarray(res.results[c][name]).reshape(-1)[:PPC]
             for c in range(N_CORES)])
        outs.append(np.ascontiguousarray(full[:, None], dtype=np.float32))
    return tuple(outs)
